# revision 2
# baseline (speedup 1.0000x reference)
"""Multi-head attention layer on 8 Trainium2 NeuronCores.

Head-parallel sharding: core h computes head h for both batches.
  - Q/K/V projections for its head (d-major QT/KT via on-chip PE transpose
    of x, token-major V).
  - scores / softmax / attn output in [q, kt] layout (softmax denominator
    comes free from the ACT pass's accum_out).
  - scores^T / exp^T in [kt, q] layout feeding the attn @ V contraction,
    with the 1/denom normalization folded into the output projection as a
    per-partition scale.
Host side: concat attn slices over heads; sum partial outputs + bias.
"""

import sys

sys.path.insert(0, "/opt/trn_rl_repo")

import numpy as np

# Problem constants (hardcoded; kernel.py must be self-contained).
B, S, D_MODEL, D_K, H = 2, 2048, 512, 64, 8
N_CORES = 8


def emit_kernel(tc, ins, outs, B, S, repeat=1):
    """Emit the per-core attention program into TileContext tc.

    ins:  dict of APs: x [B*S, 512], wq/wk/wv [512, 64], wo [64, 512]
    outs: dict of APs: attn [B, S, S], pout [B*S, 512]
    """
    import concourse.bass as bass
    from concourse import mybir
    from concourse.masks import make_identity

    nc = tc.nc
    f32 = mybir.dt.float32
    D = D_MODEL
    DK = D_K
    NT = B * S  # total tokens
    KB = D // 128  # 4 dmodel blocks
    TCH = NT // 512  # x chunks of 512 tokens
    NKT = S // 128  # kt tiles per batch
    NQC = S // 512  # q chunks per batch
    Exp = mybir.ActivationFunctionType.Exp
    scale = 1.0 / np.sqrt(DK)

    x_ap, wq_ap, wk_ap, wv_ap, wo_ap = (
        ins["x"],
        ins["wq"],
        ins["wk"],
        ins["wv"],
        ins["wo"],
    )
    attn_ap, pout_ap = outs["attn"], outs["pout"]

    from contextlib import ExitStack

    with ExitStack() as stk:
        # ---- persistent SBUF ----
        persist = stk.enter_context(tc.tile_pool(name="persist", bufs=1))
        ident = persist.tile([128, 128], f32)
        make_identity(nc, ident)
        wq_sb = persist.tile([128, KB, DK], f32, tag="wq")
        wk_sb = persist.tile([128, KB, DK], f32, tag="wk")
        wv_sb = persist.tile([128, KB, DK], f32, tag="wv")
        wo_sb = persist.tile([DK, D], f32, tag="wo")
        nc.sync.dma_start(out=wq_sb, in_=wq_ap.rearrange("(kb p) n -> p kb n", p=128))
        nc.sync.dma_start(out=wk_sb, in_=wk_ap.rearrange("(kb p) n -> p kb n", p=128))
        nc.sync.dma_start(out=wv_sb, in_=wv_ap.rearrange("(kb p) n -> p kb n", p=128))
        nc.sync.dma_start(out=wo_sb, in_=wo_ap)

        qt_all = persist.tile([DK, NT], f32, tag="qt")  # Q^T, d-major
        kt_all = persist.tile([DK, NT], f32, tag="kt")  # K^T, d-major
        v_all = persist.tile([128, NT // 128, DK], f32, tag="v")  # V, token-major
        rden = persist.tile([128, NT // 128], f32, tag="rden")  # 1/denom per q

        for rep in range(repeat):
            # ================= prologue: xT + projections =================
            with (
                tc.tile_pool(name="xin", bufs=3) as xin_pool,
                tc.tile_pool(name="xt_sb", bufs=2) as xt_sb_pool,
                tc.tile_pool(name="xt_ps", bufs=4, space="PSUM") as xt_ps_pool,
                tc.tile_pool(name="qk_ps", bufs=2, space="PSUM") as qk_ps_pool,
                tc.tile_pool(name="v_ps", bufs=2, space="PSUM") as v_ps_pool,
            ):
                for t in range(TCH):
                    x_sb = xin_pool.tile([128, 4, 512], f32)
                    nc.sync.dma_start(
                        out=x_sb,
                        in_=x_ap[t * 512 : (t + 1) * 512, :].rearrange(
                            "(i p) d -> p i d", p=128
                        ),
                    )
                    xt_sb = xt_sb_pool.tile([128, KB, 512], f32)
                    for kb in range(KB):
                        xt_ps = xt_ps_pool.tile([128, 512], f32)
                        for i in range(4):
                            # (x block)^T = lhsT.T @ I
                            nc.tensor.matmul(
                                xt_ps[:, i * 128 : (i + 1) * 128],
                                lhsT=x_sb[:, i, kb * 128 : (kb + 1) * 128],
                                rhs=ident,
                                start=True,
                                stop=True,
                            )
                        nc.vector.tensor_copy(xt_sb[:, kb, :], xt_ps)
                    # QT / KT chunks [64, 512]
                    for w_sb, acc in ((wq_sb, qt_all), (wk_sb, kt_all)):
                        qk_ps = qk_ps_pool.tile([DK, 512], f32, tag="qk")
                        for kb in range(KB):
                            nc.tensor.matmul(
                                qk_ps,
                                lhsT=w_sb[:, kb, :],
                                rhs=xt_sb[:, kb, :],
                                start=(kb == 0),
                                stop=(kb == KB - 1),
                            )
                        nc.vector.tensor_copy(
                            acc[:, t * 512 : (t + 1) * 512], qk_ps
                        )
                    # V tiles [128, 64] token-major
                    for i in range(4):
                        v_ps = v_ps_pool.tile([128, DK], f32)
                        for kb in range(KB):
                            nc.tensor.matmul(
                                v_ps,
                                lhsT=xt_sb[:, kb, i * 128 : (i + 1) * 128],
                                rhs=wv_sb[:, kb, :],
                                start=(kb == 0),
                                stop=(kb == KB - 1),
                            )
                        nc.vector.tensor_copy(v_all[:, t * 4 + i, :], v_ps)

            # ================= main attention loops =================
            with (
                tc.tile_pool(name="st_ps", bufs=2, space="PSUM") as st_ps_pool,
                tc.tile_pool(name="ctx_ps", bufs=1, space="PSUM") as ctx_ps_pool,
                tc.tile_pool(name="sc_ps", bufs=1, space="PSUM") as sc_ps_pool,
                tc.tile_pool(name="po_ps", bufs=1, space="PSUM") as po_ps_pool,
                tc.tile_pool(name="expT", bufs=3) as expT_pool,
                tc.tile_pool(name="expA", bufs=2) as expA_pool,
                tc.tile_pool(name="attn_sb", bufs=3) as attn_sb_pool,
                tc.tile_pool(name="ctx_sb", bufs=2) as ctx_sb_pool,
                tc.tile_pool(name="po_sb", bufs=2) as po_sb_pool,
                tc.tile_pool(name="den", bufs=4) as den_pool,
            ):
                for b in range(B):
                    tb = b * S  # token base of this batch
                    cb = tb // 128  # tile-column base
                    for qc in range(NQC):
                        q0 = tb + qc * 512  # first q token of chunk
                        ctx_ps = ctx_ps_pool.tile([DK, 512], f32)
                        for j in range(4):
                            # ---- phase B: NKT/4 kt-steps (scores^T -> exp^T -> ctx) ----
                            for kt in range(j * (NKT // 4), (j + 1) * (NKT // 4)):
                                st_ps = st_ps_pool.tile([128, 512], f32)
                                nc.tensor.matmul(
                                    st_ps,
                                    lhsT=kt_all[:, tb + kt * 128 : tb + (kt + 1) * 128],
                                    rhs=qt_all[:, q0 : q0 + 512],
                                    start=True,
                                    stop=True,
                                )
                                expT = expT_pool.tile([128, 512], f32)
                                nc.scalar.activation(expT, st_ps, Exp, scale=scale)
                                nc.tensor.matmul(
                                    ctx_ps,
                                    lhsT=v_all[:, cb + kt, :],
                                    rhs=expT,
                                    start=(kt == 0),
                                    stop=(kt == NKT - 1),
                                )
                            # ---- phase A: q-subtile j (scores -> exp+den -> attn) ----
                            sc_ps = sc_ps_pool.tile([128, S], f32)
                            for nn in range(NKT // 4):
                                nc.tensor.matmul(
                                    sc_ps[:, nn * 512 : (nn + 1) * 512],
                                    lhsT=qt_all[
                                        :, q0 + j * 128 : q0 + (j + 1) * 128
                                    ],
                                    rhs=kt_all[:, tb + nn * 512 : tb + (nn + 1) * 512],
                                    start=True,
                                    stop=True,
                                )
                            exp_a = expA_pool.tile([128, S], f32)
                            den = den_pool.tile([128, 1], f32)
                            nc.scalar.activation(
                                exp_a, sc_ps, Exp, scale=scale, accum_out=den
                            )
                            col = cb + qc * 4 + j
                            nc.vector.reciprocal(rden[:, col : col + 1], den)
                            attn_sb = attn_sb_pool.tile([128, S], f32)
                            nc.vector.tensor_scalar_mul(
                                attn_sb, exp_a, rden[:, col : col + 1]
                            )
                            nc.sync.dma_start(
                                out=attn_ap[
                                    b, qc * 512 + j * 128 : qc * 512 + (j + 1) * 128, :
                                ],
                                in_=attn_sb,
                            )
                        # ---- output projection for this q chunk ----
                        ctx_sb = ctx_sb_pool.tile([DK, 512], f32)
                        nc.vector.tensor_copy(ctx_sb, ctx_ps)
                        po_sb = po_sb_pool.tile([128, 4, D], f32)
                        for j in range(4):
                            po_ps = po_ps_pool.tile([128, D], f32)
                            nc.tensor.matmul(
                                po_ps,
                                lhsT=ctx_sb[:, j * 128 : (j + 1) * 128],
                                rhs=wo_sb,
                                start=True,
                                stop=True,
                            )
                            col = cb + qc * 4 + j
                            nc.vector.tensor_scalar_mul(
                                po_sb[:, j, :], po_ps, rden[:, col : col + 1]
                            )
                        nc.sync.dma_start(
                            out=pout_ap[q0 : q0 + 512, :].rearrange(
                                "(j p) d -> p j d", p=128
                            ),
                            in_=po_sb,
                        )


def build_nc(B, S, repeat=1, n_cores=N_CORES):
    import concourse.tile as tile
    from concourse import bacc, mybir

    f32 = mybir.dt.float32
    nc = bacc.Bacc("TRN2", target_bir_lowering=False, debug=False, num_devices=n_cores)
    NT = B * S
    ins = {
        "x": nc.dram_tensor("x", [NT, D_MODEL], f32, kind="ExternalInput").ap(),
        "wq": nc.dram_tensor("wq", [D_MODEL, D_K], f32, kind="ExternalInput").ap(),
        "wk": nc.dram_tensor("wk", [D_MODEL, D_K], f32, kind="ExternalInput").ap(),
        "wv": nc.dram_tensor("wv", [D_MODEL, D_K], f32, kind="ExternalInput").ap(),
        "wo": nc.dram_tensor("wo", [D_K, D_MODEL], f32, kind="ExternalInput").ap(),
    }
    outs = {
        "attn": nc.dram_tensor("attn", [B, S, S], f32, kind="ExternalOutput").ap(),
        "pout": nc.dram_tensor("pout", [NT, D_MODEL], f32, kind="ExternalOutput").ap(),
    }
    with tile.TileContext(nc) as tc:
        emit_kernel(tc, ins, outs, B, S, repeat=repeat)
    nc.compile()
    return nc


def make_runner(nc, n_cores):
    """Cached jitted SPMD callable for a compiled Bass program."""
    import jax
    from jax.sharding import Mesh, PartitionSpec
    from jax.experimental.shard_map import shard_map
    from concourse import mybir
    from concourse.bass2jax import (
        _bass_exec_p,
        install_neuronx_cc_hook,
        partition_id_tensor,
    )

    install_neuronx_cc_hook()
    partition_name = nc.partition_id_tensor.name if nc.partition_id_tensor else None
    in_names, out_names, out_avals, zero_outs = [], [], [], []
    for alloc in nc.m.functions[0].allocations:
        if not isinstance(alloc, mybir.MemoryLocationSet):
            continue
        name = alloc.memorylocations[0].name
        if alloc.kind == "ExternalInput":
            if name != partition_name:
                in_names.append(name)
        elif alloc.kind == "ExternalOutput":
            out_names.append(name)
            shape = tuple(alloc.tensor_shape)
            dtype = mybir.dt.np(alloc.dtype)
            out_avals.append(jax.core.ShapedArray(shape, dtype))
            zero_outs.append(np.zeros(shape, dtype))
    n_params = len(in_names)
    all_names = list(in_names) + list(out_names)
    if partition_name is not None:
        all_names.append(partition_name)

    def _body(*args):
        operands = list(args)
        if partition_name is not None:
            operands.append(partition_id_tensor())
        return tuple(
            _bass_exec_p.bind(
                *operands,
                out_avals=tuple(out_avals),
                in_names=tuple(all_names),
                out_names=tuple(out_names),
                lowering_input_output_aliases=(),
                sim_require_finite=True,
                sim_require_nnan=True,
                nc=nc,
            )
        )

    devices = jax.devices()[:n_cores]
    mesh = Mesh(np.asarray(devices), ("core",))
    in_specs = (PartitionSpec("core"),) * (n_params + len(out_names))
    out_specs = (PartitionSpec("core"),) * len(out_names)
    fn = jax.jit(
        shard_map(
            _body, mesh=mesh, in_specs=in_specs, out_specs=out_specs, check_rep=False
        ),
        keep_unused=True,
    )
    return fn, in_names, out_names, out_avals


def shard_inputs(x, Wq, Wk, Wv, Wo):
    """Per-core input dicts: core h gets full x + head-h weight slices."""
    x2 = np.ascontiguousarray(np.asarray(x, np.float32).reshape(B * S, D_MODEL))
    maps = []
    for h in range(N_CORES):
        sl = slice(h * D_K, (h + 1) * D_K)
        maps.append(
            {
                "x": x2,
                "wq": np.ascontiguousarray(np.asarray(Wq, np.float32)[:, sl]),
                "wk": np.ascontiguousarray(np.asarray(Wk, np.float32)[:, sl]),
                "wv": np.ascontiguousarray(np.asarray(Wv, np.float32)[:, sl]),
                "wo": np.ascontiguousarray(np.asarray(Wo, np.float32)[sl, :]),
            }
        )
    return maps


_CACHE = {}


def _get_compiled(repeat=1):
    key = repeat
    if key not in _CACHE:
        nc = build_nc(B, S, repeat=repeat)
        _CACHE[key] = (nc,) + tuple(make_runner(nc, N_CORES))
    return _CACHE[key]


def run_on_device(in_maps, repeat=1):
    """Run the SPMD kernel; returns (list of per-core output dicts, fn, args)."""
    import jax

    nc, fn, in_names, out_names, out_avals = _get_compiled(repeat)
    concat_in = [
        np.concatenate([m[n] for m in in_maps], axis=0) for n in in_names
    ]
    concat_zeros = [
        np.zeros((N_CORES * a.shape[0],) + a.shape[1:], a.dtype) for a in out_avals
    ]
    args = [jax.device_put(a) for a in concat_in + concat_zeros]
    out = jax.block_until_ready(fn(*args))
    results = [
        {
            n: np.asarray(out[i]).reshape((N_CORES,) + out_avals[i].shape)[c]
            for i, n in enumerate(out_names)
        }
        for c in range(N_CORES)
    ]
    return results, fn, args


def kernel(x, Wq, Wk, Wv, Wo, bo):
    in_maps = shard_inputs(x, Wq, Wk, Wv, Wo)
    results, _, _ = run_on_device(in_maps)
    attn = np.stack([results[h]["attn"] for h in range(N_CORES)], axis=1)
    out = np.sum([results[h]["pout"] for h in range(N_CORES)], axis=0)
    out = (out + np.asarray(bo, np.float32)).reshape(B, S, D_MODEL)
    return out, attn
